# revision 1
# baseline (speedup 1.0000x reference)
"""Causal self-attention with RoPE on 8 TRN2 NeuronCores.

Sharding: pure data parallel over batch B=8 (one batch element per core,
weights replicated, no collectives).

Per-core dataflow (everything "transposed" so softmax reductions and biases
land on friendly axes):
  xT = x^T                          via PE transpose          [C, T]
  q^T,k^T = W_qk^T @ x + b          PE (W stationary)         [ch, T]
  v natural = x @ W_v + b_v         PE (xT stationary)        [T, ch]
  RoPE(q,k)                         PE rotation matmul + DVE  in place
  s^T = k @ q^T (per head)          PE, K=64                  [Tk, Tq]
  p = exp(s/8) * causal_mask        ACT exp + DVE mask
  [y'; r]^T = [v, 1]^T @ p          PE, K=128 accumulation    [65, Tq]
  y^T = y'^T * (1/r)                DVE (+ gpsimd broadcast)
  out = y @ W_proj + b              PE (yT stationary)        [T, C]

Matmuls run in float32r (fp32 data, 12-bit-mantissa multiply) = 4x fp32 rate.
"""
import sys

sys.path.insert(0, "/opt/trn_rl_repo")

import numpy as np

B, T, C = 8, 1024, 768
H, D = 12, 64
N_CORES = 8
KC = C // 128  # 6 K-chunks of the C contraction
NT = T // 128  # 8 T-chunks

_prog = None  # cached compiled Bass program
_EXP_FUNC = "Exp"  # timing experiments may override


def _emit_body(nc, tc, dr, phases=(1, 2, 3)):
    """Emit one full forward pass. dr = dict of DRAM tensors."""
    from concourse import mybir

    F32 = mybir.dt.float32
    F32R = mybir.dt.float32r
    BF16 = mybir.dt.bfloat16
    AFT = mybir.ActivationFunctionType

    with (
        tc.tile_pool(name="persist", bufs=1) as pp,
    ):
        # persistent tensors
        qkT = pp.tile([128, 12, T], F32R, tag="qkT")  # 0-5: q pairs, 6-11: k pairs
        v_sb = pp.tile([128, NT, H, 65], F32R, tag="v")  # v natural + ones col
        yT = pp.tile([128, KC, T], F32R, tag="yT")
        cos_sb = pp.tile([128, T], F32, tag="cos")
        sin_sb = pp.tile([128, T], F32, tag="sin")
        rt_sb = pp.tile([128, 128], F32R, tag="rt")
        idn_sb = pp.tile([128, 128], F32, tag="idn")
        mask_sb = pp.tile([128, 128], F32R, tag="mask")
        mneg_sb = pp.tile([128, 128], BF16, tag="mneg")
        idnr_sb = pp.tile([128, 128], BF16, tag="idnr")
        bqk_sb = pp.tile([128, 12], F32, tag="bqk")
        bv_sb = pp.tile([1, C], F32R, tag="bv")
        bp_sb = pp.tile([1, C], F32R, tag="bp")
        ones_sb = pp.tile([1, 128], F32R, tag="ones")

        nc.sync.dma_start(out=idn_sb[:], in_=dr["idn"][:])
        nc.sync.dma_start(out=cos_sb[:], in_=dr["cosT"][:])
        nc.sync.dma_start(out=sin_sb[:], in_=dr["sinT"][:])
        nc.sync.dma_start(out=rt_sb[:], in_=dr["rt"][:].bitcast(F32R))
        nc.sync.dma_start(out=mask_sb[:], in_=dr["mask"][:].bitcast(F32R))
        nc.sync.dma_start(out=mneg_sb[:], in_=dr["mnegb"][:])
        nc.sync.dma_start(out=idnr_sb[:], in_=dr["idnb"][:])
        nc.sync.dma_start(out=bqk_sb[:], in_=dr["bqk"][:])
        nc.sync.dma_start(out=bv_sb[:], in_=dr["bv"][:].bitcast(F32R))
        nc.sync.dma_start(out=bp_sb[:], in_=dr["bp"][:].bitcast(F32R))
        onesF = pp.tile([128, 128], F32, tag="onesF")
        nc.vector.memset(onesF[:], 1.0)
        nc.vector.tensor_copy(ones_sb[:], onesF[0:1, :])
        ones64 = pp.tile([128, 64], F32R, tag="ones64")
        nc.vector.tensor_copy(ones64[:], onesF[:, 0:64])
        for t in range(NT):
            nc.vector.tensor_copy(
                v_sb[:, t, :, 64:65],
                onesF[:, 0:12].rearrange("p (h o) -> p h o", h=12),
            )

        # ---------------- Phase A: transpose x, qkv, rope ----------------
        if 1 not in phases:
            return
        with (
            tc.tile_pool(name="pa_sb", bufs=2) as pa,
            tc.tile_pool(name="pa_xt", bufs=1) as paxt,
            tc.tile_pool(name="pa_ps", bufs=2, space="PSUM") as pap,
            tc.tile_pool(name="pa_mm", bufs=3, space="PSUM") as pam,
            tc.tile_pool(name="pa_tmp", bufs=3) as pat,
        ):
            xT = paxt.tile([128, KC, T], F32R, tag="xT")
            for t in range(NT):
                xn = pa.tile([128, C], F32, tag="xn", bufs=4)
                nc.sync.dma_start(out=xn[:], in_=dr["x"][t * 128 : (t + 1) * 128, :])
                for c in range(KC):
                    ptr = pap.tile([128, 128], F32, tag="tr")
                    nc.tensor.transpose(
                        ptr[:], xn[:, c * 128 : (c + 1) * 128], idn_sb[:]
                    )
                    nc.scalar.activation(
                        xT[:, c, t * 128 : (t + 1) * 128], ptr[:], AFT.Identity
                    )

            def _rope(i):
                t1 = pat.tile([128, T], F32, tag="t1", bufs=2)
                nc.vector.tensor_mul(t1[:], qkT[:, i, :], cos_sb[:])
                for pj in range(2):
                    w = slice(pj * 512, (pj + 1) * 512)
                    rp = pap.tile([128, 512], F32, tag="rot")
                    nc.tensor.matmul(
                        rp[:], rt_sb[:], qkT[:, i, w], start=True, stop=True
                    )
                    t2 = pat.tile([128, 512], F32, tag="t2")
                    nc.vector.tensor_mul(t2[:], rp[:], sin_sb[:, w])
                    nc.vector.tensor_add(qkT[:, i, w], t1[:, w], t2[:])

            # qkv in 6 column groups of 384 (W_attn streamed per group)
            wa_r = dr["wa"][:].bitcast(F32R).rearrange("(kc p) n -> p kc n", p=128)
            for g in range(6):
                wt = pa.tile([128, KC, 384], F32R, tag="walt")
                nc.sync.dma_start(out=wt[:], in_=wa_r[:, :, g * 384 : (g + 1) * 384])
                if g < 4:  # q/k output chunks m = 3g..3g+2
                    for pj in range(2):
                        for mi in range(3):
                            m = 3 * g + mi
                            w = slice(pj * 512, (pj + 1) * 512)
                            ps = pam.tile([128, 512], F32, tag="mm")
                            for kc in range(KC):
                                nc.tensor.matmul(
                                    ps[:],
                                    wt[:, kc, mi * 128 : (mi + 1) * 128],
                                    xT[:, kc, w],
                                    start=(kc == 0),
                                    stop=(kc == KC - 1),
                                )
                            nc.scalar.activation(
                                qkT[:, m, w],
                                ps[:],
                                AFT.Identity,
                                bias=bqk_sb[:, m : m + 1],
                            )
                    for mi in range(3):
                        _rope(3 * g + mi)
                else:  # v columns: 384-wide piece covers 6 heads
                    vg = g - 4
                    h0 = 6 * vg
                    for t in range(NT):
                        ps = pam.tile([128, 384], F32, tag="mm")
                        for kc in range(KC):
                            nc.tensor.matmul(
                                ps[:],
                                xT[:, kc, t * 128 : (t + 1) * 128],
                                wt[:, kc, :],
                                start=(kc == 0),
                                stop=False,
                            )
                        nc.tensor.matmul(
                            ps[:],
                            ones_sb[:],
                            bv_sb[:, vg * 384 : (vg + 1) * 384],
                            start=False,
                            stop=True,
                        )
                        nc.vector.tensor_copy(
                            v_sb[:, t, h0 : h0 + 6, 0:64],
                            ps[:].rearrange("p (h d) -> p h d", h=6),
                        )


        # ---------------- Phase B: attention per head ----------------
        if 2 not in phases:
            return
        bc_pool_cm = tc.tile_pool(name="pbc_wp", bufs=1)
        bc_pool = bc_pool_cm.__enter__()
        wp = bc_pool.tile([128, KC, C], F32R, tag="wp")
        nc.sync.dma_start(
            out=wp[:],
            in_=dr["wp"][:].bitcast(F32R).rearrange("(kc p) n -> p kc n", p=128),
        )
        with (
            tc.tile_pool(name="pb_es", bufs=8) as pbe,
            tc.tile_pool(name="pb_sc", bufs=4) as pbs,
            tc.tile_pool(name="pb_st", bufs=2, space="PSUM") as pbst,
            tc.tile_pool(name="pb_yp", bufs=3, space="PSUM") as pbyp,
            tc.tile_pool(name="pb_rb", bufs=1, space="PSUM") as pbrb,
        ):
            for hp in range(6):
                qv, kv = hp, 6 + hp
                es_store = {}
                for tkc in range(NT):
                    lo = 128 * tkc
                    width = T - lo
                    sts = {}
                    for hh in range(2):  # adjacent K=64 MMs -> row-group overlap
                        b0 = 64 * hh
                        st = pbst.tile([128, 1024], F32, tag="st")
                        sts[hh] = st
                        off = 0
                        while off < width:
                            wdt = min(512, width - off)
                            nc.tensor.matmul(
                                st[:, off : off + wdt],
                                qkT[b0 : b0 + 64, kv, lo : lo + 128],
                                qkT[b0 : b0 + 64, qv, lo + off : lo + off + wdt],
                                start=True,
                                stop=not (off == 0),
                            )
                            if off == 0:
                                nc.tensor.matmul(
                                    st[:, 0:128],
                                    idnr_sb[:],
                                    mneg_sb[:],
                                    start=False,
                                    stop=True,
                                )
                            off += wdt
                    for hh in range(2):
                        st = sts[hh]
                        for pj in range(2):
                            w0 = 512 * pj
                            if lo >= w0 + 512:
                                continue
                            plo = max(w0, lo)
                            wdt = w0 + 512 - plo
                            es = pbe.tile([128, 512], F32R, tag="es", bufs=24)
                            nc.scalar.activation(
                                es[:, :wdt],
                                st[:, plo - lo : plo - lo + wdt],
                                getattr(AFT, _EXP_FUNC),
                                scale=0.125,
                            )
                            es_store[(hh, tkc, pj)] = (es, plo, wdt)
                for pj in range(2):
                    w0 = 512 * pj
                    tkcs = [k for k in range(NT) if 128 * k < w0 + 512]
                    for hh in range(2):
                        h = 2 * hp + hh
                        yp = pbyp.tile([65, 512], F32, tag="yp")
                        for j, tkc in enumerate(tkcs):
                            es, plo, wdt = es_store[(hh, tkc, pj)]
                            nc.tensor.matmul(
                                yp[:, plo - w0 : plo - w0 + wdt],
                                v_sb[:, tkc, h, :],
                                es[:, :wdt],
                                start=(j == 0),
                                stop=(j == len(tkcs) - 1),
                            )
                        # normalize: y = y' / r  (r = row 64 of yp)
                        rs = pbs.tile([128, 512], F32R, tag="rs")
                        nc.vector.tensor_copy(rs[64:65, :], yp[64:65, :])
                        rbp = pbrb.tile([64, 512], F32, tag="rbp")
                        nc.tensor.matmul(
                            rbp[:], ones64[64:65, :], rs[64:65, :],
                            start=True, stop=True,
                        )
                        rbf = pbs.tile([64, 512], F32, tag="rbf")
                        nc.vector.reciprocal_approx_fast(out=rbf[:], in_=rbp[:])
                        if hh == 0:
                            nc.vector.tensor_mul(
                                yT[0:64, hp, w0 : w0 + 512], yp[0:64, :], rbf[:]
                            )
                        else:
                            ys = pbs.tile([64, 512], F32R, tag="ys")
                            nc.vector.tensor_mul(ys[:], yp[0:64, :], rbf[:])
                            nc.sync.dma_start(
                                out=yT[64:128, hp, w0 : w0 + 512], in_=ys[:]
                            )

        # ---------------- Phase C: output projection ----------------
        if 3 not in phases:
            bc_pool_cm.__exit__(None, None, None)
            return
        with (
            tc.tile_pool(name="pc_ob", bufs=3) as pco,
            tc.tile_pool(name="pc_ps", bufs=3, space="PSUM") as pcp,
        ):
            for m in range(NT):
                osb = pco.tile([128, C], F32, tag="ob")
                for piece in range(2):
                    pw = slice(piece * 384, (piece + 1) * 384)
                    po = pcp.tile([128, 384], F32, tag="po")
                    for kc in range(KC):
                        nc.tensor.matmul(
                            po[:],
                            yT[:, kc, m * 128 : (m + 1) * 128],
                            wp[:, kc, pw],
                            start=(kc == 0),
                            stop=False,
                        )
                    nc.tensor.matmul(
                        po[:], ones_sb[:], bp_sb[:, pw], start=False, stop=True
                    )
                    nc.vector.tensor_copy(osb[:, pw], po[:])
                nc.sync.dma_start(out=dr["out"][m * 128 : (m + 1) * 128, :], in_=osb[:])
        bc_pool_cm.__exit__(None, None, None)


def _build_program(loop_n=None, phases=(1, 2, 3)):
    import concourse.bacc as bacc
    import concourse.tile as tile
    from concourse import mybir

    F32 = mybir.dt.float32

    nc = bacc.Bacc(None, target_bir_lowering=False, debug=False)

    dr = {
        "x": nc.dram_tensor("x", [T, C], F32, kind="ExternalInput"),
        "wa": nc.dram_tensor("wa", [C, 3 * C], F32, kind="ExternalInput"),
        "bqk": nc.dram_tensor("bqk", [128, 12], F32, kind="ExternalInput"),
        "bv": nc.dram_tensor("bv", [1, C], F32, kind="ExternalInput"),
        "wp": nc.dram_tensor("wp", [C, C], F32, kind="ExternalInput"),
        "bp": nc.dram_tensor("bp", [1, C], F32, kind="ExternalInput"),
        "cosT": nc.dram_tensor("cosT", [128, T], F32, kind="ExternalInput"),
        "sinT": nc.dram_tensor("sinT", [128, T], F32, kind="ExternalInput"),
        "rt": nc.dram_tensor("rt", [128, 128], F32, kind="ExternalInput"),
        "idn": nc.dram_tensor("idn", [128, 128], F32, kind="ExternalInput"),
        "mask": nc.dram_tensor("mask", [128, 128], F32, kind="ExternalInput"),
        "mnegb": nc.dram_tensor("mnegb", [128, 128], mybir.dt.bfloat16, kind="ExternalInput"),
        "idnb": nc.dram_tensor("idnb", [128, 128], mybir.dt.bfloat16, kind="ExternalInput"),
        "out": nc.dram_tensor("out", [T, C], F32, kind="ExternalOutput"),
    }

    with tile.TileContext(nc) as tc:
        if loop_n is None:
            _emit_body(nc, tc, dr, phases)
        else:
            with tc.For_i(0, loop_n, 1):
                _emit_body(nc, tc, dr, phases)

    nc.compile()
    return nc


def _host_constants():
    """Constant tables shipped to every core."""
    inv_freq = (1.0 / (10000.0 ** (np.arange(0, D, 2, dtype=np.float32) / D))).astype(
        np.float32
    )
    tpos = np.arange(T, dtype=np.float32)
    freqs = tpos[None, :] * inv_freq[:, None]  # [32, T]
    cos32 = np.cos(freqs).astype(np.float32)
    sin32 = np.sin(freqs).astype(np.float32)
    cosT = np.repeat(cos32, 2, axis=0)  # [64, T], channel d -> freq d//2
    sinT = np.repeat(sin32, 2, axis=0)
    cosT = np.concatenate([cosT, cosT], axis=0)  # [128, T]: two head copies
    sinT = np.concatenate([sinT, sinT], axis=0)

    # rotation matrix: rot = R @ q with rot[2i] = -q[2i+1], rot[2i+1] = q[2i]
    R = np.zeros((128, 128), dtype=np.float32)
    idx = np.arange(0, 128, 2)
    R[idx, idx + 1] = -1.0
    R[idx + 1, idx] = 1.0
    RT = np.ascontiguousarray(R.T)

    idn = np.eye(128, dtype=np.float32)
    # causal mask in s^T orientation: keep tq_rel >= tk (upper incl diag)
    mask = np.triu(np.ones((128, 128), dtype=np.float32))
    # additive mask: -1e5 (pre-scale) where tq_rel < tk so exp(0.125*s) == 0
    mneg = (-1.0e5 * np.tril(np.ones((128, 128), dtype=np.float32), k=-1)).astype(np.float32)
    import ml_dtypes
    mneg_b = mneg.astype(ml_dtypes.bfloat16)
    idn_b = idn.astype(ml_dtypes.bfloat16)
    return cosT, sinT, RT, idn, mask, mneg_b, idn_b


def _input_maps(x, W_attn, b_attn, W_proj, b_proj):
    cosT, sinT, RT, idn, mask, mneg_b, idn_b = _host_constants()
    shared = {
        "wa": np.ascontiguousarray(W_attn),
        "bqk": np.ascontiguousarray(b_attn[: 2 * C].reshape(12, 128).T),
        "bv": np.ascontiguousarray(b_attn[2 * C :].reshape(1, C)),
        "wp": np.ascontiguousarray(W_proj),
        "bp": np.ascontiguousarray(b_proj.reshape(1, C)),
        "cosT": cosT,
        "sinT": sinT,
        "rt": RT,
        "idn": idn,
        "mask": mask,
        "mnegb": mneg_b,
        "idnb": idn_b,
    }
    return [dict(shared, x=np.ascontiguousarray(x[b])) for b in range(B)]


def kernel(x, W_attn, b_attn, W_proj, b_proj):
    global _prog
    from concourse.bass_utils import run_bass_kernel_spmd

    if _prog is None:
        _prog = _build_program()

    x = np.asarray(x, dtype=np.float32)
    W_attn = np.asarray(W_attn, dtype=np.float32)
    b_attn = np.asarray(b_attn, dtype=np.float32)
    W_proj = np.asarray(W_proj, dtype=np.float32)
    b_proj = np.asarray(b_proj, dtype=np.float32)

    in_maps = _input_maps(x, W_attn, b_attn, W_proj, b_proj)
    res = run_bass_kernel_spmd(_prog, in_maps, list(range(N_CORES)))
    out = np.stack([res.results[b]["out"] for b in range(B)], axis=0)
    return out.astype(np.float32)



# revision 82
# speedup vs baseline: 1.2791x; 1.2791x over previous
"""Causal self-attention with RoPE on 8 TRN2 NeuronCores.

Sharding: pure data parallel over batch B=8 (one batch element per core,
weights replicated, no collectives).

Per-core dataflow (host pre-transposes x and pre-packs weights in bf16),
software-pipelined per head-pair hp so ACT exp work for pair hp-1 overlaps
the PE matmul streams for pair hp:

  xT (host)                          DMA                      [C, T] bf16
  q^T,k^T = W_qk^T @ x + b           PE bf16 (W stationary)   [ch, T] f32r
  v natural = x @ W_v + b            PE bf16, DVE bias-add    [T, ch] bf16
  RoPE(q,k)                          PE rot + GPSIMD/DVE      in place
  s^T = k @ q^T (per head)           PE f32r, K=64            [Tk, Tq]
  p = exp(s/8) with causal mask      ACT exp (mask via PE)    bf16
  [y'; r]^T = [v, 1]^T @ p           PE bf16, K=128 accum     [65, Tq]
  y^T = y'^T * (1/r)                 DVE recip + PE bcast     bf16
  out = y @ W_proj + b               PE bf16, DVE bias-add    [T, C] f32

All wide matmuls stream 1 col/cycle; no f32r matmul narrower than 256
(4x penalty); biases folded into ACT/DVE copies; exp merged into one
activation per (head, key-chunk). PSUM: poolX 2x[128,1024] (4 banks) +
yp 2 + rbp 2 = 8 banks.
"""
import sys

sys.path.insert(0, "/opt/trn_rl_repo")

import numpy as np

B, T, C = 8, 1024, 768
H, D = 12, 64
N_CORES = 8
KC = C // 128  # 6 K-chunks of the C contraction
NT = T // 128  # 8 T-chunks
TP = T + 128  # qkT padded free size so narrow s-pieces can read 256 wide

# wa group offsets (in columns of the host-packed [128, KC, ...] layout):
# group 0: vA (384 cols), group 1: vB (384), groups 2..7: pair hp (256)
_WAOFF = [0, 384, 768, 1024, 1280, 1536, 1792, 2048]  # start col of each group
_WATOT = 2304  # total packed columns

_prog = None  # cached compiled Bass program
_DEBUG = False  # add intermediate-dump DMAs (qkT, v_sb, yT)


def _emit_body(nc, tc, dr, phases=(1, 2, 3)):
    """Emit one full forward pass. dr = dict of DRAM tensors."""
    from concourse import mybir

    F32 = mybir.dt.float32
    F32R = mybir.dt.float32r
    BF16 = mybir.dt.bfloat16
    AFT = mybir.ActivationFunctionType

    with (
        tc.tile_pool(name="persist", bufs=1) as pp,
        tc.tile_pool(name="wts", bufs=1) as pw,
        tc.tile_pool(name="ps5", bufs=4, space="PSUM") as ps5,
        tc.tile_pool(name="stw", bufs=2, space="PSUM") as stw,
        tc.tile_pool(name="ptmp", bufs=3) as pat,
        tc.tile_pool(name="pes", bufs=12) as pes,
        tc.tile_pool(name="pesw", bufs=12) as pesw,
        tc.tile_pool(name="pnrm", bufs=3) as pbs,
    ):
        # persistent tensors
        qkT = pp.tile([128, 12, T], BF16, tag="qkT")  # 0-5: q pairs, 6-11: k
        v_sb = pp.tile([128, NT, H, 65], BF16, tag="v")  # v natural + ones col
        yT = pp.tile([128, KC, T], BF16, tag="yT")
        xt_sb = pp.tile([128, KC, T], BF16, tag="xt")
        # packed consts: cos | signed-sin | mneg | idn | ones
        cbf = pp.tile([128, 2 * T + 320], BF16, tag="cbf")
        cf32 = pp.tile([128, 12], F32, tag="cf32")  # qk bias per pair-channel
        cos_sb = cbf[:, 0:T]
        sin_sb = cbf[:, T : 2 * T]
        mneg_sb = cbf[:, 2 * T : 2 * T + 128]
        idnr_sb = cbf[:, 2 * T + 128 : 2 * T + 256]
        ones_b = cbf[:, 2 * T + 256 : 2 * T + 320]
        bqk_sb = cf32
        bvbc_sb = pp.tile([128, C], BF16, tag="bvbc")
        bpbc_sb = pp.tile([128, C], F32, tag="bpbc")
        wp_sb = pp.tile([128, KC, C], BF16, tag="wp")

        xt_r = dr["xt"][:].rearrange("p (kc t) -> p kc t", kc=KC)

        def _load_wa(tag, g, cols):
            wt = pw.tile([128, KC, cols], BF16, tag=tag, bufs=3)
            start = KC * _WAOFF[g]
            nc.sync.dma_start(
                out=wt[:],
                in_=dr["wa"][:, start : start + KC * cols].rearrange(
                    "p (kc n) -> p kc n", kc=KC
                ),
            )
            return wt

        # --- init DMAs in first-use order (SP queue is FIFO) ---
        nc.sync.dma_start(out=xt_sb[:, :, 0:512], in_=xt_r[:, :, 0:512])
        wtp = {0: _load_wa("wtp", 2, 256)}
        nc.sync.dma_start(out=xt_sb[:, :, 512:1024], in_=xt_r[:, :, 512:1024])
        nc.sync.dma_start(out=cf32[:], in_=dr["cf32"][:])
        nc.sync.dma_start(out=cbf[:], in_=dr["cbf"][:])
        wtv = [_load_wa("wtv", 0, 384), _load_wa("wtv", 1, 384)]
        wtp[1] = _load_wa("wtp", 3, 256)
        nc.sync.dma_start(out=bvbc_sb[:], in_=dr["bvbc"][:])
        nc.sync.dma_start(out=bpbc_sb[:], in_=dr["bpbc"][:])
        # ones column of v via host DMA (DVE memset miscompiled on HW)
        nc.sync.dma_start(
            out=v_sb[:, :, :, 64:65],
            in_=dr["vones"][:].rearrange("p (a b o) -> p a b o", a=NT, b=H),
        )


        def emit_qk(i):
            wt = wtp[i]
            for which, m in ((0, i), (1, 6 + i)):
                pss = [
                    ps5.tile([128, 512], F32, tag="ps5", name=f"ps_{which}_{pj}")
                    for pj in range(2)
                ]
                if i == 0 and which == 0:
                    # startup: all pj0 matmuls first (xt half 1 still in DMA)
                    order = [(kc, pj) for pj in range(2) for kc in range(KC)]
                else:
                    # both pj share the stationary weight
                    order = [(kc, pj) for kc in range(KC) for pj in range(2)]
                for kc, pj in order:
                    nc.tensor.matmul(
                        pss[pj][:],
                        wt[:, kc, which * 128 : which * 128 + 128],
                        xt_sb[:, kc, pj * 512 : (pj + 1) * 512],
                        start=(kc == 0),
                        stop=(kc == KC - 1),
                    )
                for pj in range(2):
                    w = slice(pj * 512, (pj + 1) * 512)
                    if which == 0:  # q copies on ACT (bias fused)
                        nc.scalar.activation(
                            qkT[:, m, w], pss[pj][:], AFT.Identity,
                            bias=bqk_sb[:, m : m + 1],
                        )
                    else:  # k copies on DVE
                        nc.vector.tensor_scalar_add(
                            qkT[:, m, w], pss[pj][:], bqk_sb[:, m : m + 1]
                        )

        # rope rotation = adjacent-partition swap; the sign lives in the
        # host-packed signed sin table, so no PE rotation matmul is needed.
        # (GPSIMD is avoided: its tensor ops produce garbage on the first
        # post-load execution.)
        swap_mask = [i ^ 1 for i in range(32)]

        def emit_rope(i):
            for m in (i, 6 + i):
                shf = pat.tile([128, T], BF16, tag="shf", bufs=2)
                nc.vector.stream_shuffle(shf[:], qkT[:, m, :], swap_mask)
                t1 = pat.tile([128, T], BF16, tag="t1", bufs=2)
                nc.vector.tensor_mul(t1[:], qkT[:, m, :], cos_sb[:])
                t2 = pat.tile([128, T], BF16, tag="t2", bufs=2)
                nc.vector.tensor_mul(t2[:], shf[:], sin_sb[:])
                nc.vector.tensor_add(qkT[:, m, :], t1[:], t2[:])

        es_store = {}

        def emit_s(i, tkcs):
            qv, kv = i, 6 + i
            for tkc in tkcs:
                lo = 128 * tkc
                width = T - lo
                wide = width > 512
                for hh in range(2):
                    b0 = 64 * hh
                    if wide:
                        st = stw.tile([128, 1024], F32, tag="stw")
                    else:
                        st = ps5.tile([128, 512], F32, tag="ps5")
                    for off in range(0, width, 512):
                        valid_w = min(512, width - off)
                        nc.tensor.matmul(
                            st[:, off : off + valid_w],
                            qkT[b0 : b0 + 64, kv, lo : lo + 128],
                            qkT[b0 : b0 + 64, qv, lo + off : lo + off + valid_w],
                            start=True,
                            stop=not (off == 0),
                        )
                        if off == 0:  # causal mask add on the diagonal block
                            nc.tensor.matmul(
                                st[:, 0:128],
                                idnr_sb,
                                mneg_sb,
                                start=False,
                                stop=True,
                            )
                    if wide:
                        es = pesw.tile([128, 1024], BF16, tag="esw", name="esw")
                    else:
                        es = pes.tile([128, 512], BF16, tag="es", name="es")
                    nc.scalar.activation(
                        es[:, :width], st[:, :width], AFT.Exp, scale=0.125
                    )
                    es_store[(hh, tkc)] = es

        def emit_v():
            for vg in range(2):
                wt = wtv[vg]
                for t in range(NT):
                    ps = ps5.tile([128, 512], F32, tag="ps5")
                    for kc in range(KC):
                        nc.tensor.matmul(
                            ps[:, 0:384],
                            xt_sb[:, kc, t * 128 : (t + 1) * 128],
                            wt[:, kc, :],
                            start=(kc == 0),
                            stop=(kc == KC - 1),
                        )
                    nc.vector.tensor_add(
                        v_sb[:, t, 6 * vg : 6 * vg + 6, 0:64],
                        ps[:, 0:384].rearrange("p (h d) -> p h d", h=6),
                        bvbc_sb[:, vg * 384 : (vg + 1) * 384].rearrange(
                            "p (h d) -> p h d", h=6
                        ),
                    )

        def emit_y(i):
            for pj in range(2):
                w0 = 512 * pj
                tkcs = [k for k in range(NT) if 128 * k < w0 + 512]
                for hh in range(2):
                    h = 2 * i + hh
                    yp = ps5.tile([128, 512], F32, tag="ps5")
                    for j, tkc in enumerate(tkcs):
                        lo = 128 * tkc
                        plo = max(w0, lo)
                        wdt = w0 + 512 - plo
                        es = es_store[(hh, tkc)]
                        nc.tensor.matmul(
                            yp[0:65, plo - w0 : plo - w0 + wdt],
                            v_sb[:, tkc, h, :],
                            es[:, plo - lo : plo - lo + wdt],
                            start=(j == 0),
                            stop=(j == len(tkcs) - 1),
                        )
                    # normalize: y = y' / r  (r = row 64 of yp):
                    # r -> SBUF f32r, broadcast to 64 rows on PE, reciprocal
                    # moves it back to SBUF, multiply
                    rs = pbs.tile([128, 512], BF16, tag="rs")
                    nc.scalar.activation(
                        rs[64:65, :], yp[64:65, :], AFT.Identity
                    )
                    rbp = stw.tile([128, 1024], F32, tag="stw")
                    nc.tensor.matmul(
                        rbp[0:64, 0:512],
                        ones_b[64:65, :],
                        rs[64:65, :],
                        start=True,
                        stop=True,
                    )
                    rbf = pbs.tile([64, 512], F32, tag="rbf")
                    nc.vector.reciprocal_approx_fast(
                        out=rbf[:], in_=rbp[0:64, 0:512]
                    )
                    if hh == 0:
                        nc.vector.tensor_mul(
                            yT[0:64, i, w0 : w0 + 512], yp[0:64, :], rbf[:]
                        )
                    else:
                        ys = pbs.tile([64, 512], BF16, tag="ys")
                        nc.vector.tensor_mul(ys[:], yp[0:64, :], rbf[:])
                        nc.sync.dma_start(
                            out=yT[64:128, i, w0 : w0 + 512], in_=ys[:]
                        )

        # ---------------- pipelined qkv + attention ----------------
        if 1 not in phases:
            return
        emit_qk(0)
        emit_rope(0)
        for i in range(1, 7):
            if i + 1 < 6:
                wtp[i + 1] = _load_wa("wtp", 2 + (i + 1), 256)
            if i == 4:
                nc.sync.dma_start(
                    out=wp_sb[:],
                    in_=dr["wp"][:].rearrange("p (kc n) -> p kc n", kc=KC),
                )
            if 2 in phases:
                emit_s(i - 1, range(0, 2))
            if i == 1:
                emit_v()
            if i < 6:
                emit_qk(i)
                emit_rope(i)
            if 2 in phases:
                emit_s(i - 1, range(2, NT))
                emit_y(i - 1)

        if _DEBUG:
            nc.sync.dma_start(
                out=dr["dqk"][:], in_=qkT[:].rearrange("p a b -> p (a b)")
            )
            nc.sync.dma_start(
                out=dr["dv"][:], in_=v_sb[:].rearrange("p a b c -> p (a b c)")
            )
            nc.sync.dma_start(
                out=dr["dyt"][:], in_=yT[:].rearrange("p a b -> p (a b)")
            )

        # ---------------- output projection ----------------
        if 3 not in phases:
            return
        with tc.tile_pool(name="pc_ob", bufs=3) as pco:
            for m in range(NT):
                osb = pco.tile([128, C], F32, tag="ob")
                for piece in range(2):
                    pw_ = slice(piece * 384, (piece + 1) * 384)
                    po = ps5.tile([128, 512], F32, tag="ps5")
                    for kc in range(KC):
                        nc.tensor.matmul(
                            po[:, 0:384],
                            yT[:, kc, m * 128 : (m + 1) * 128],
                            wp_sb[:, kc, pw_],
                            start=(kc == 0),
                            stop=(kc == KC - 1),
                        )
                    nc.vector.tensor_add(osb[:, pw_], po[:, 0:384], bpbc_sb[:, pw_])
                    nc.sync.dma_start(
                        out=dr["out"][m * 128 : (m + 1) * 128, pw_],
                        in_=osb[:, pw_],
                    )


def _build_program(loop_n=None, phases=(1, 2, 3)):
    import concourse.bacc as bacc
    import concourse.tile as tile
    from concourse import mybir

    F32 = mybir.dt.float32
    BF16 = mybir.dt.bfloat16

    nc = bacc.Bacc(None, target_bir_lowering=False, debug=False)

    dr = {
        "xt": nc.dram_tensor("xt", [128, KC * T], BF16, kind="ExternalInput"),
        "wa": nc.dram_tensor("wa", [128, KC * _WATOT], BF16, kind="ExternalInput"),
        "bvbc": nc.dram_tensor("bvbc", [128, C], BF16, kind="ExternalInput"),
        "wp": nc.dram_tensor("wp", [128, KC * C], BF16, kind="ExternalInput"),
        "bpbc": nc.dram_tensor("bpbc", [128, C], F32, kind="ExternalInput"),
        "cbf": nc.dram_tensor("cbf", [128, 2 * T + 320], BF16, kind="ExternalInput"),
        "cf32": nc.dram_tensor("cf32", [128, 12], F32, kind="ExternalInput"),
        "vones": nc.dram_tensor("vones", [128, NT * H], BF16, kind="ExternalInput"),
        "out": nc.dram_tensor("out", [T, C], F32, kind="ExternalOutput"),
    }
    if _DEBUG:
        dr["dqk"] = nc.dram_tensor("dqk", [128, 12 * T], BF16, kind="ExternalOutput")
        dr["dv"] = nc.dram_tensor("dv", [128, NT * H * 65], BF16, kind="ExternalOutput")
        dr["dyt"] = nc.dram_tensor("dyt", [128, KC * T], BF16, kind="ExternalOutput")

    with tile.TileContext(nc) as tc:
        if loop_n is None:
            _emit_body(nc, tc, dr, phases)
        else:
            with tc.For_i(0, loop_n, 1):
                _emit_body(nc, tc, dr, phases)

    nc.compile()
    return nc


def _host_constants():
    """Constant tables shipped to every core."""
    inv_freq = (1.0 / (10000.0 ** (np.arange(0, D, 2, dtype=np.float32) / D))).astype(
        np.float32
    )
    tpos = np.arange(T, dtype=np.float32)
    freqs = tpos[None, :] * inv_freq[:, None]  # [32, T]
    cos32 = np.cos(freqs).astype(np.float32)
    sin32 = np.sin(freqs).astype(np.float32)
    cosT = np.repeat(cos32, 2, axis=0)  # [64, T], channel d -> freq d//2
    sinT = np.repeat(sin32, 2, axis=0)
    cosT = np.concatenate([cosT, cosT], axis=0)  # [128, T]: two head copies
    sinT = np.concatenate([sinT, sinT], axis=0)

    # rotation = adjacent-row swap; fold the signs into the sin table:
    # rot[2i] = -q[2i+1]*sin, rot[2i+1] = +q[2i]*sin
    sinS = sinT.copy()
    sinS[0::2, :] *= -1.0

    import ml_dtypes

    # additive mask in s^T orientation: -1e5 (pre-scale) where tq_rel < tk
    mneg = (-1.0e5 * np.tril(np.ones((128, 128), dtype=np.float32), k=-1)).astype(
        ml_dtypes.bfloat16
    )
    idn_b = np.eye(128, dtype=np.float32).astype(ml_dtypes.bfloat16)
    return cosT, sinS, mneg, idn_b


def _input_maps(x, W_attn, b_attn, W_proj, b_proj):
    import ml_dtypes

    BF = ml_dtypes.bfloat16
    cosT, sinS, mneg_b, idn_b = _host_constants()

    # wa: [C, 3C] -> [128, kc, n], columns packed as [vA | vB | pair0..pair5]
    wa = W_attn.reshape(KC, 128, 3 * C).transpose(1, 0, 2)  # [128, kc, 3C]
    groups = [wa[:, :, 2 * C : 2 * C + 384], wa[:, :, 2 * C + 384 : 3 * C]]
    for hp in range(6):
        groups.append(wa[:, :, hp * 128 : (hp + 1) * 128])  # q pair
        groups.append(wa[:, :, C + hp * 128 : C + (hp + 1) * 128])  # k pair
    # merge each pair's q+k into one 256-col group
    packed = [groups[0], groups[1]] + [
        np.concatenate([groups[2 + 2 * hp], groups[3 + 2 * hp]], axis=2)
        for hp in range(6)
    ]
    wa_g = np.concatenate([g.reshape(128, -1) for g in packed], axis=1)
    assert wa_g.shape[1] == KC * _WATOT

    wp = W_proj.reshape(KC, 128, C).transpose(1, 0, 2).reshape(128, KC * C)

    bqk = b_attn[: 2 * C].reshape(12, 128).T.astype(np.float32)
    cbf = np.concatenate(
        [
            cosT.astype(BF),
            sinS.astype(BF),
            mneg_b,
            idn_b,
            np.ones((128, 64), BF),
        ],
        axis=1,
    )
    cf32 = np.ascontiguousarray(bqk)
    shared = {
        "wa": np.ascontiguousarray(wa_g.astype(BF)),
        "bvbc": np.ascontiguousarray(
            np.broadcast_to(b_attn[2 * C :].astype(BF), (128, C))
        ),
        "wp": np.ascontiguousarray(wp.astype(BF)),
        "bpbc": np.ascontiguousarray(np.broadcast_to(b_proj, (128, C))),
        "cbf": np.ascontiguousarray(cbf),
        "cf32": cf32,
        "vones": np.ones((128, NT * H), dtype=BF),
    }
    out = []
    for b in range(B):
        xt = (
            x[b].T.reshape(KC, 128, T).transpose(1, 0, 2).reshape(128, KC * T)
        )  # [128, KC*T]
        out.append(dict(shared, xt=np.ascontiguousarray(xt.astype(BF))))
    return out


def kernel(x, W_attn, b_attn, W_proj, b_proj):
    global _prog
    from concourse.bass_utils import run_bass_kernel_spmd

    if _prog is None:
        _prog = _build_program()

    x = np.asarray(x, dtype=np.float32)
    W_attn = np.asarray(W_attn, dtype=np.float32)
    b_attn = np.asarray(b_attn, dtype=np.float32)
    W_proj = np.asarray(W_proj, dtype=np.float32)
    b_proj = np.asarray(b_proj, dtype=np.float32)

    in_maps = _input_maps(x, W_attn, b_attn, W_proj, b_proj)
    # first post-load execution shows cold-start wobble in some ucode
    # engines; run once to warm up, return the steady-state result
    run_bass_kernel_spmd(_prog, in_maps, list(range(N_CORES)))
    res = run_bass_kernel_spmd(_prog, in_maps, list(range(N_CORES)))
    out = np.stack([res.results[b]["out"] for b in range(B)], axis=0)
    return out.astype(np.float32)


# revision 90
# speedup vs baseline: 47.9902x; 37.5174x over previous
"""Causal self-attention with RoPE on 8 TRN2 NeuronCores.

Sharding: pure data parallel over batch B=8 (one batch element per core,
weights replicated, no collectives).

Per-core dataflow (host pre-transposes x and pre-packs weights in bf16),
software-pipelined per head-pair hp so ACT exp work for pair hp-1 overlaps
the PE matmul streams for pair hp:

  xT (host)                          DMA                      [C, T] bf16
  q^T,k^T = W_qk^T @ x + b           PE bf16 (W stationary)   [ch, T] f32r
  v natural = x @ W_v + b            PE bf16, DVE bias-add    [T, ch] bf16
  RoPE(q,k)                          PE rot + GPSIMD/DVE      in place
  s^T = k @ q^T (per head)           PE f32r, K=64            [Tk, Tq]
  p = exp(s/8) with causal mask      ACT exp (mask via PE)    bf16
  [y'; r]^T = [v, 1]^T @ p           PE bf16, K=128 accum     [65, Tq]
  y^T = y'^T * (1/r)                 DVE recip + PE bcast     bf16
  out = y @ W_proj + b               PE bf16, DVE bias-add    [T, C] f32

All wide matmuls stream 1 col/cycle; no f32r matmul narrower than 256
(4x penalty); biases folded into ACT/DVE copies; exp merged into one
activation per (head, key-chunk). PSUM: poolX 2x[128,1024] (4 banks) +
yp 2 + rbp 2 = 8 banks.
"""
import sys

sys.path.insert(0, "/opt/trn_rl_repo")

import numpy as np

B, T, C = 8, 1024, 768
H, D = 12, 64
N_CORES = 8
KC = C // 128  # 6 K-chunks of the C contraction
NT = T // 128  # 8 T-chunks
TP = T + 128  # qkT padded free size so narrow s-pieces can read 256 wide

# wa group offsets (in columns of the host-packed [128, KC, ...] layout):
# group 0: vA (384 cols), group 1: vB (384), groups 2..7: pair hp (256)
_WAOFF = [0, 384, 768, 1024, 1280, 1536, 1792, 2048]  # start col of each group
_WATOT = 2304  # total packed columns

_prog = None  # cached compiled Bass program
_DEBUG = False  # add intermediate-dump DMAs (qkT, v_sb, yT)


def _emit_body(nc, tc, dr, phases=(1, 2, 3)):
    """Emit one full forward pass. dr = dict of DRAM tensors."""
    from concourse import mybir

    F32 = mybir.dt.float32
    F32R = mybir.dt.float32r
    BF16 = mybir.dt.bfloat16
    AFT = mybir.ActivationFunctionType

    with (
        tc.tile_pool(name="persist", bufs=1) as pp,
        tc.tile_pool(name="wts", bufs=1) as pw,
        tc.tile_pool(name="ps5", bufs=4, space="PSUM") as ps5,
        tc.tile_pool(name="stw", bufs=2, space="PSUM") as stw,
        tc.tile_pool(name="ptmp", bufs=3) as pat,
        tc.tile_pool(name="pes", bufs=12) as pes,
        tc.tile_pool(name="pesw", bufs=12) as pesw,
        tc.tile_pool(name="pnrm", bufs=3) as pbs,
    ):
        # persistent tensors
        qkT = pp.tile([128, 12, T], BF16, tag="qkT")  # 0-5: q pairs, 6-11: k
        v_sb = pp.tile([128, NT, H, 65], BF16, tag="v")  # v natural + ones col
        yT = pp.tile([128, KC, T], BF16, tag="yT")
        xt_sb = pp.tile([128, KC, T], BF16, tag="xt")
        # packed consts: cos | signed-sin | mneg | idn | ones
        cbf = pp.tile([128, 2 * T + 320], BF16, tag="cbf")
        cf32 = pp.tile([128, 12], F32, tag="cf32")  # qk bias per pair-channel
        cos_sb = cbf[:, 0:T]
        sin_sb = cbf[:, T : 2 * T]
        mneg_sb = cbf[:, 2 * T : 2 * T + 128]
        idnr_sb = cbf[:, 2 * T + 128 : 2 * T + 256]
        ones_b = cbf[:, 2 * T + 256 : 2 * T + 320]
        bqk_sb = cf32
        bvbc_sb = pp.tile([128, C], BF16, tag="bvbc")
        bpbc_sb = pp.tile([128, C], F32, tag="bpbc")
        wp_sb = pp.tile([128, KC, C], BF16, tag="wp")

        xt_r = dr["xt"][:].rearrange("p (kc t) -> p kc t", kc=KC)

        def _load_wa(tag, g, cols):
            wt = pw.tile([128, KC, cols], BF16, tag=tag, bufs=3)
            start = KC * _WAOFF[g]
            nc.sync.dma_start(
                out=wt[:],
                in_=dr["wa"][:, start : start + KC * cols].rearrange(
                    "p (kc n) -> p kc n", kc=KC
                ),
            )
            return wt

        # --- init DMAs in first-use order (SP queue is FIFO) ---
        nc.sync.dma_start(out=xt_sb[:, :, 0:512], in_=xt_r[:, :, 0:512])
        wtp = {0: _load_wa("wtp", 2, 256)}
        nc.sync.dma_start(out=xt_sb[:, :, 512:1024], in_=xt_r[:, :, 512:1024])
        nc.sync.dma_start(out=cf32[:], in_=dr["cf32"][:])
        nc.sync.dma_start(out=cbf[:], in_=dr["cbf"][:])
        wtv = [_load_wa("wtv", 0, 384), _load_wa("wtv", 1, 384)]
        wtp[1] = _load_wa("wtp", 3, 256)
        nc.sync.dma_start(out=bvbc_sb[:], in_=dr["bvbc"][:])
        nc.sync.dma_start(out=bpbc_sb[:], in_=dr["bpbc"][:])
        # ones column of v via host DMA (DVE memset miscompiled on HW)
        nc.sync.dma_start(
            out=v_sb[:, :, :, 64:65],
            in_=dr["vones"][:].rearrange("p (a b o) -> p a b o", a=NT, b=H),
        )


        def emit_qk(i):
            wt = wtp[i]
            for which, m in ((0, i), (1, 6 + i)):
                pss = [
                    ps5.tile([128, 512], F32, tag="ps5", name=f"ps_{which}_{pj}")
                    for pj in range(2)
                ]
                if i == 0 and which == 0:
                    # startup: all pj0 matmuls first (xt half 1 still in DMA)
                    order = [(kc, pj) for pj in range(2) for kc in range(KC)]
                else:
                    # both pj share the stationary weight
                    order = [(kc, pj) for kc in range(KC) for pj in range(2)]
                for kc, pj in order:
                    nc.tensor.matmul(
                        pss[pj][:],
                        wt[:, kc, which * 128 : which * 128 + 128],
                        xt_sb[:, kc, pj * 512 : (pj + 1) * 512],
                        start=(kc == 0),
                        stop=(kc == KC - 1),
                    )
                for pj in range(2):
                    w = slice(pj * 512, (pj + 1) * 512)
                    if which == 0:  # q copies on ACT (bias fused)
                        nc.scalar.activation(
                            qkT[:, m, w], pss[pj][:], AFT.Identity,
                            bias=bqk_sb[:, m : m + 1],
                        )
                    else:  # k copies on DVE
                        nc.vector.tensor_scalar_add(
                            qkT[:, m, w], pss[pj][:], bqk_sb[:, m : m + 1]
                        )

        # rope rotation = adjacent-partition swap; the sign lives in the
        # host-packed signed sin table, so no PE rotation matmul is needed.
        # (GPSIMD is avoided: its tensor ops produce garbage on the first
        # post-load execution.)
        swap_mask = [i ^ 1 for i in range(32)]

        def emit_rope(i):
            for m in (i, 6 + i):
                shf = pat.tile([128, T], BF16, tag="shf", bufs=2)
                nc.vector.stream_shuffle(shf[:], qkT[:, m, :], swap_mask)
                t1 = pat.tile([128, T], BF16, tag="t1", bufs=2)
                nc.vector.tensor_mul(t1[:], qkT[:, m, :], cos_sb[:])
                t2 = pat.tile([128, T], BF16, tag="t2", bufs=2)
                nc.vector.tensor_mul(t2[:], shf[:], sin_sb[:])
                nc.vector.tensor_add(qkT[:, m, :], t1[:], t2[:])

        es_store = {}

        def emit_s(i, tkcs):
            qv, kv = i, 6 + i
            for tkc in tkcs:
                lo = 128 * tkc
                width = T - lo
                wide = width > 512
                for hh in range(2):
                    b0 = 64 * hh
                    if wide:
                        st = stw.tile([128, 1024], F32, tag="stw")
                    else:
                        st = ps5.tile([128, 512], F32, tag="ps5")
                    for off in range(0, width, 512):
                        valid_w = min(512, width - off)
                        nc.tensor.matmul(
                            st[:, off : off + valid_w],
                            qkT[b0 : b0 + 64, kv, lo : lo + 128],
                            qkT[b0 : b0 + 64, qv, lo + off : lo + off + valid_w],
                            start=True,
                            stop=not (off == 0),
                        )
                        if off == 0:  # causal mask add on the diagonal block
                            nc.tensor.matmul(
                                st[:, 0:128],
                                idnr_sb,
                                mneg_sb,
                                start=False,
                                stop=True,
                            )
                    if wide:
                        es = pesw.tile([128, 1024], BF16, tag="esw", name="esw")
                    else:
                        es = pes.tile([128, 512], BF16, tag="es", name="es")
                    nc.scalar.activation(
                        es[:, :width], st[:, :width], AFT.Exp, scale=0.125
                    )
                    es_store[(hh, tkc)] = es

        def emit_v():
            for vg in range(2):
                wt = wtv[vg]
                for t in range(NT):
                    ps = ps5.tile([128, 512], F32, tag="ps5")
                    for kc in range(KC):
                        nc.tensor.matmul(
                            ps[:, 0:384],
                            xt_sb[:, kc, t * 128 : (t + 1) * 128],
                            wt[:, kc, :],
                            start=(kc == 0),
                            stop=(kc == KC - 1),
                        )
                    nc.vector.tensor_add(
                        v_sb[:, t, 6 * vg : 6 * vg + 6, 0:64],
                        ps[:, 0:384].rearrange("p (h d) -> p h d", h=6),
                        bvbc_sb[:, vg * 384 : (vg + 1) * 384].rearrange(
                            "p (h d) -> p h d", h=6
                        ),
                    )

        def emit_y(i, pjs=(0, 1)):
            for pj in pjs:
                w0 = 512 * pj
                tkcs = [k for k in range(NT) if 128 * k < w0 + 512]
                for hh in range(2):
                    h = 2 * i + hh
                    yp = ps5.tile([128, 512], F32, tag="ps5")
                    for j, tkc in enumerate(tkcs):
                        lo = 128 * tkc
                        plo = max(w0, lo)
                        wdt = w0 + 512 - plo
                        es = es_store[(hh, tkc)]
                        nc.tensor.matmul(
                            yp[0:65, plo - w0 : plo - w0 + wdt],
                            v_sb[:, tkc, h, :],
                            es[:, plo - lo : plo - lo + wdt],
                            start=(j == 0),
                            stop=(j == len(tkcs) - 1),
                        )
                    # normalize: y = y' / r  (r = row 64 of yp):
                    # r -> SBUF f32r, broadcast to 64 rows on PE, reciprocal
                    # moves it back to SBUF, multiply
                    rs = pbs.tile([128, 512], BF16, tag="rs")
                    nc.scalar.activation(
                        rs[64:65, :], yp[64:65, :], AFT.Identity
                    )
                    rbp = stw.tile([128, 1024], F32, tag="stw")
                    nc.tensor.matmul(
                        rbp[0:64, 0:512],
                        ones_b[64:65, :],
                        rs[64:65, :],
                        start=True,
                        stop=True,
                    )
                    rbf = pbs.tile([64, 512], F32, tag="rbf")
                    nc.vector.reciprocal_approx_fast(
                        out=rbf[:], in_=rbp[0:64, 0:512]
                    )
                    if hh == 0:
                        nc.vector.tensor_mul(
                            yT[0:64, i, w0 : w0 + 512], yp[0:64, :], rbf[:]
                        )
                    else:
                        ys = pbs.tile([64, 512], BF16, tag="ys")
                        nc.vector.tensor_mul(ys[:], yp[0:64, :], rbf[:])
                        nc.sync.dma_start(
                            out=yT[64:128, i, w0 : w0 + 512], in_=ys[:]
                        )

        # ---------------- pipelined qkv + attention ----------------
        if 1 not in phases:
            return
        emit_qk(0)
        emit_rope(0)
        for i in range(1, 7):
            if i + 1 < 6:
                wtp[i + 1] = _load_wa("wtp", 2 + (i + 1), 256)
            if i == 4:
                nc.sync.dma_start(
                    out=wp_sb[:],
                    in_=dr["wp"][:].rearrange("p (kc n) -> p kc n", kc=KC),
                )
            if 2 in phases:
                emit_s(i - 1, range(0, 2))
            if i == 1:
                emit_v()
            if i < 6:
                emit_qk(i)
                emit_rope(i)
            if 2 in phases:
                emit_s(i - 1, range(2, NT))
                emit_y(i - 1)

        if _DEBUG:
            nc.sync.dma_start(
                out=dr["dqk"][:], in_=qkT[:].rearrange("p a b -> p (a b)")
            )
            nc.sync.dma_start(
                out=dr["dv"][:], in_=v_sb[:].rearrange("p a b c -> p (a b c)")
            )
            nc.sync.dma_start(
                out=dr["dyt"][:], in_=yT[:].rearrange("p a b -> p (a b)")
            )

        # ---------------- output projection ----------------
        if 3 not in phases:
            return
        with tc.tile_pool(name="pc_ob", bufs=3) as pco:
            for m in range(NT):
                osb = pco.tile([128, C], F32, tag="ob")
                for piece in range(2):
                    pw_ = slice(piece * 384, (piece + 1) * 384)
                    po = ps5.tile([128, 512], F32, tag="ps5")
                    for kc in range(KC):
                        nc.tensor.matmul(
                            po[:, 0:384],
                            yT[:, kc, m * 128 : (m + 1) * 128],
                            wp_sb[:, kc, pw_],
                            start=(kc == 0),
                            stop=(kc == KC - 1),
                        )
                    nc.vector.tensor_add(osb[:, pw_], po[:, 0:384], bpbc_sb[:, pw_])
                    nc.sync.dma_start(
                        out=dr["out"][m * 128 : (m + 1) * 128, pw_],
                        in_=osb[:, pw_],
                    )


def _build_program(loop_n=None, phases=(1, 2, 3)):
    import concourse.bacc as bacc
    import concourse.tile as tile
    from concourse import mybir

    F32 = mybir.dt.float32
    BF16 = mybir.dt.bfloat16

    nc = bacc.Bacc(None, target_bir_lowering=False, debug=False)

    dr = {
        "xt": nc.dram_tensor("xt", [128, KC * T], BF16, kind="ExternalInput"),
        "wa": nc.dram_tensor("wa", [128, KC * _WATOT], BF16, kind="ExternalInput"),
        "bvbc": nc.dram_tensor("bvbc", [128, C], BF16, kind="ExternalInput"),
        "wp": nc.dram_tensor("wp", [128, KC * C], BF16, kind="ExternalInput"),
        "bpbc": nc.dram_tensor("bpbc", [128, C], F32, kind="ExternalInput"),
        "cbf": nc.dram_tensor("cbf", [128, 2 * T + 320], BF16, kind="ExternalInput"),
        "cf32": nc.dram_tensor("cf32", [128, 12], F32, kind="ExternalInput"),
        "vones": nc.dram_tensor("vones", [128, NT * H], BF16, kind="ExternalInput"),
        "out": nc.dram_tensor("out", [T, C], F32, kind="ExternalOutput"),
    }
    if _DEBUG:
        dr["dqk"] = nc.dram_tensor("dqk", [128, 12 * T], BF16, kind="ExternalOutput")
        dr["dv"] = nc.dram_tensor("dv", [128, NT * H * 65], BF16, kind="ExternalOutput")
        dr["dyt"] = nc.dram_tensor("dyt", [128, KC * T], BF16, kind="ExternalOutput")

    with tile.TileContext(nc) as tc:
        if loop_n is None:
            _emit_body(nc, tc, dr, phases)
        else:
            with tc.For_i(0, loop_n, 1):
                _emit_body(nc, tc, dr, phases)

    nc.compile()
    return nc


def _host_constants():
    """Constant tables shipped to every core."""
    inv_freq = (1.0 / (10000.0 ** (np.arange(0, D, 2, dtype=np.float32) / D))).astype(
        np.float32
    )
    tpos = np.arange(T, dtype=np.float32)
    freqs = tpos[None, :] * inv_freq[:, None]  # [32, T]
    cos32 = np.cos(freqs).astype(np.float32)
    sin32 = np.sin(freqs).astype(np.float32)
    cosT = np.repeat(cos32, 2, axis=0)  # [64, T], channel d -> freq d//2
    sinT = np.repeat(sin32, 2, axis=0)
    cosT = np.concatenate([cosT, cosT], axis=0)  # [128, T]: two head copies
    sinT = np.concatenate([sinT, sinT], axis=0)

    # rotation = adjacent-row swap; fold the signs into the sin table:
    # rot[2i] = -q[2i+1]*sin, rot[2i+1] = +q[2i]*sin
    sinS = sinT.copy()
    sinS[0::2, :] *= -1.0

    import ml_dtypes

    # additive mask in s^T orientation: -1e5 (pre-scale) where tq_rel < tk
    mneg = (-1.0e5 * np.tril(np.ones((128, 128), dtype=np.float32), k=-1)).astype(
        ml_dtypes.bfloat16
    )
    idn_b = np.eye(128, dtype=np.float32).astype(ml_dtypes.bfloat16)
    return cosT, sinS, mneg, idn_b


def _input_maps(x, W_attn, b_attn, W_proj, b_proj):
    import ml_dtypes

    BF = ml_dtypes.bfloat16
    cosT, sinS, mneg_b, idn_b = _host_constants()

    # wa: [C, 3C] -> [128, kc, n], columns packed as [vA | vB | pair0..pair5]
    wa = W_attn.reshape(KC, 128, 3 * C).transpose(1, 0, 2)  # [128, kc, 3C]
    groups = [wa[:, :, 2 * C : 2 * C + 384], wa[:, :, 2 * C + 384 : 3 * C]]
    for hp in range(6):
        groups.append(wa[:, :, hp * 128 : (hp + 1) * 128])  # q pair
        groups.append(wa[:, :, C + hp * 128 : C + (hp + 1) * 128])  # k pair
    # merge each pair's q+k into one 256-col group
    packed = [groups[0], groups[1]] + [
        np.concatenate([groups[2 + 2 * hp], groups[3 + 2 * hp]], axis=2)
        for hp in range(6)
    ]
    wa_g = np.concatenate([g.reshape(128, -1) for g in packed], axis=1)
    assert wa_g.shape[1] == KC * _WATOT

    wp = W_proj.reshape(KC, 128, C).transpose(1, 0, 2).reshape(128, KC * C)

    bqk = b_attn[: 2 * C].reshape(12, 128).T.astype(np.float32)
    cbf = np.concatenate(
        [
            cosT.astype(BF),
            sinS.astype(BF),
            mneg_b,
            idn_b,
            np.ones((128, 64), BF),
        ],
        axis=1,
    )
    cf32 = np.ascontiguousarray(bqk)
    shared = {
        "wa": np.ascontiguousarray(wa_g.astype(BF)),
        "bvbc": np.ascontiguousarray(
            np.broadcast_to(b_attn[2 * C :].astype(BF), (128, C))
        ),
        "wp": np.ascontiguousarray(wp.astype(BF)),
        "bpbc": np.ascontiguousarray(np.broadcast_to(b_proj, (128, C))),
        "cbf": np.ascontiguousarray(cbf),
        "cf32": cf32,
        "vones": np.ones((128, NT * H), dtype=BF),
    }
    out = []
    for b in range(B):
        xt = (
            x[b].T.reshape(KC, 128, T).transpose(1, 0, 2).reshape(128, KC * T)
        )  # [128, KC*T]
        out.append(dict(shared, xt=np.ascontiguousarray(xt.astype(BF))))
    return out


def kernel(x, W_attn, b_attn, W_proj, b_proj):
    global _prog
    from concourse.bass_utils import run_bass_kernel_spmd

    if _prog is None:
        _prog = _build_program()

    x = np.asarray(x, dtype=np.float32)
    W_attn = np.asarray(W_attn, dtype=np.float32)
    b_attn = np.asarray(b_attn, dtype=np.float32)
    W_proj = np.asarray(W_proj, dtype=np.float32)
    b_proj = np.asarray(b_proj, dtype=np.float32)

    in_maps = _input_maps(x, W_attn, b_attn, W_proj, b_proj)
    # first post-load execution shows cold-start wobble in some ucode
    # engines; run once to warm up, return the steady-state result
    run_bass_kernel_spmd(_prog, in_maps, list(range(N_CORES)))
    res = run_bass_kernel_spmd(_prog, in_maps, list(range(N_CORES)))
    out = np.stack([res.results[b]["out"] for b in range(B)], axis=0)
    return out.astype(np.float32)


# revision 107
# speedup vs baseline: 48.0854x; 1.0020x over previous
"""Causal self-attention with RoPE on 8 TRN2 NeuronCores.

Sharding: pure data parallel over batch B=8 (one batch element per core,
weights replicated, no collectives).

Per-core dataflow, all matmuls bf16 with fp32 PSUM accumulation. The host
pre-transposes x, pre-packs W_attn by head-pair, pre-broadcasts biases and
pre-signs the sin table. The emission is software-pipelined per head-pair
hp: ACT exponentials for pair hp-1 stream underneath the qkv matmuls of
pair hp.

  xT (host transpose)                DMA                       [C, T]
  q^T,k^T = W_qk^T @ x + b           PE (W stationary), copies [ch, T]
                                     with fused bias on DVE
  v natural = x @ W_v + b            PE, DVE bias-add          [T, ch]
  RoPE(q,k) in place                 DVE only: stream_shuffle pair-swap,
                                     q*cos + swap(q)*signed_sin
  s^T = k @ q^T (per head)           PE K=64                   [Tk, Tq]
  p = exp(s/8), causal mask added    ACT exp <=1024 wide; mask via one
    on the diagonal block            PE matmul (idn @ mneg) into PSUM
  [y'; r]^T = [v, 1]^T @ p           PE K=128 accum            [65, Tq]
  y^T = y'^T * (1/r)                 r: ACT copy -> PE ones-broadcast ->
                                     DVE reciprocal -> DVE mul (odd heads
                                     reach partitions 64:127 via SB->SB DMA)
  out = y @ W_proj + b               PE, DVE bias-add          [T, C] f32

PSUM (8 banks exactly): ps5 shared 4x[128,512] rotation (qkv/v/narrow
scores/y'/proj accumulators) + stw 2x[128,1024] (wide scores, shared with
the reciprocal-broadcast tiles).

HW notes: GPSIMD tensor ops corrupt the first post-load execution (kernel()
therefore runs the NEFF twice and returns the steady-state result); DVE
memsets on this toolchain miscompile (constants arrive via host DMAs);
fp32r was abandoned entirely so no producer-rounding verifier rules apply.
"""
import sys

sys.path.insert(0, "/opt/trn_rl_repo")

import numpy as np

B, T, C = 8, 1024, 768
H, D = 12, 64
N_CORES = 8
KC = C // 128  # 6 K-chunks of the C contraction
NT = T // 128  # 8 T-chunks

# wa group offsets (in columns of the host-packed [128, KC, ...] layout):
# group 0: vA (384 cols), group 1: vB (384), groups 2..7: pair hp (256)
_WAOFF = [0, 384, 768, 1024, 1280, 1536, 1792, 2048]  # start col of each group
_WATOT = 2304  # total packed columns

_prog = None  # cached compiled Bass program
_DEBUG = False  # add intermediate-dump DMAs (qkT, v_sb, yT)


def _emit_body(nc, tc, dr, phases=(1, 2, 3)):
    """Emit one full forward pass. dr = dict of DRAM tensors."""
    from concourse import mybir

    F32 = mybir.dt.float32
    F32R = mybir.dt.float32r
    BF16 = mybir.dt.bfloat16
    AFT = mybir.ActivationFunctionType

    with (
        tc.tile_pool(name="persist", bufs=1) as pp,
        tc.tile_pool(name="wts", bufs=1) as pw,
        tc.tile_pool(name="ps5", bufs=4, space="PSUM") as ps5,
        tc.tile_pool(name="stw", bufs=2, space="PSUM") as stw,
        tc.tile_pool(name="ptmp", bufs=3) as pat,
        tc.tile_pool(name="pes", bufs=12) as pes,
        tc.tile_pool(name="pesw", bufs=12) as pesw,
        tc.tile_pool(name="pnrm", bufs=3) as pbs,
    ):
        # persistent tensors
        qkT = pp.tile([128, 12, T], BF16, tag="qkT")  # 0-5: q pairs, 6-11: k
        v_sb = pp.tile([128, NT, H, 65], BF16, tag="v")  # v natural + ones col
        yT = pp.tile([128, KC, T], BF16, tag="yT")
        xt_sb = pp.tile([128, KC, T], BF16, tag="xt")
        # packed consts: cos | signed-sin | mneg | idn | ones
        cbf = pp.tile([128, 2 * T + 320], BF16, tag="cbf")
        cf32 = pp.tile([128, 12], F32, tag="cf32")  # qk bias per pair-channel
        cos_sb = cbf[:, 0:T]
        sin_sb = cbf[:, T : 2 * T]
        mneg_sb = cbf[:, 2 * T : 2 * T + 128]
        idnr_sb = cbf[:, 2 * T + 128 : 2 * T + 256]
        ones_b = cbf[:, 2 * T + 256 : 2 * T + 320]
        bqk_sb = cf32
        bvbc_sb = pp.tile([128, C], BF16, tag="bvbc")
        bpbc_sb = pp.tile([128, C], F32, tag="bpbc")
        wp_sb = pp.tile([128, KC, C], BF16, tag="wp")

        xt_r = dr["xt"][:].rearrange("p (kc t) -> p kc t", kc=KC)

        def _load_wa(tag, g, cols):
            wt = pw.tile([128, KC, cols], BF16, tag=tag, bufs=3)
            start = KC * _WAOFF[g]
            nc.sync.dma_start(
                out=wt[:],
                in_=dr["wa"][:, start : start + KC * cols].rearrange(
                    "p (kc n) -> p kc n", kc=KC
                ),
            )
            return wt

        # --- init DMAs in first-use order (SP queue is FIFO) ---
        nc.sync.dma_start(out=xt_sb[:, :, 0:512], in_=xt_r[:, :, 0:512])
        wtp = {0: _load_wa("wtp", 2, 256)}
        nc.sync.dma_start(out=xt_sb[:, :, 512:1024], in_=xt_r[:, :, 512:1024])
        nc.sync.dma_start(out=cf32[:], in_=dr["cf32"][:])
        nc.sync.dma_start(out=cbf[:], in_=dr["cbf"][:])
        wtv = [_load_wa("wtv", 0, 384), _load_wa("wtv", 1, 384)]
        wtp[1] = _load_wa("wtp", 3, 256)
        nc.sync.dma_start(out=bvbc_sb[:], in_=dr["bvbc"][:])
        nc.sync.dma_start(out=bpbc_sb[:], in_=dr["bpbc"][:])
        # ones column of v via host DMA (DVE memset miscompiled on HW)
        nc.sync.dma_start(
            out=v_sb[:, :, :, 64:65],
            in_=dr["vones"][:].rearrange("p (a b o) -> p a b o", a=NT, b=H),
        )


        def emit_qk(i):
            wt = wtp[i]
            for which, m in ((0, i), (1, 6 + i)):
                pss = [
                    ps5.tile([128, 512], F32, tag="ps5", name=f"ps_{which}_{pj}")
                    for pj in range(2)
                ]
                if i == 0 and which == 0:
                    # startup: all pj0 matmuls first (xt half 1 still in DMA)
                    order = [(kc, pj) for pj in range(2) for kc in range(KC)]
                else:
                    # both pj share the stationary weight
                    order = [(kc, pj) for kc in range(KC) for pj in range(2)]
                for kc, pj in order:
                    nc.tensor.matmul(
                        pss[pj][:],
                        wt[:, kc, which * 128 : which * 128 + 128],
                        xt_sb[:, kc, pj * 512 : (pj + 1) * 512],
                        start=(kc == 0),
                        stop=(kc == KC - 1),
                    )
                for pj in range(2):
                    w = slice(pj * 512, (pj + 1) * 512)
                    # both q and k copies on DVE: ACT then runs only the exp
                    # stream + rs copies, finishing ~2us earlier per
                    # iteration, which un-gates the pj1 y matmuls and the
                    # normalize tail that otherwise stalls the next
                    # iteration's first score tile
                    nc.vector.tensor_scalar_add(
                        qkT[:, m, w], pss[pj][:], bqk_sb[:, m : m + 1]
                    )

        # rope rotation = adjacent-partition swap; the sign lives in the
        # host-packed signed sin table, so no PE rotation matmul is needed.
        # (GPSIMD is avoided: its tensor ops produce garbage on the first
        # post-load execution.)
        swap_mask = [i ^ 1 for i in range(32)]

        def emit_rope(i):
            for m in (i, 6 + i):
                shf = pat.tile([128, T], BF16, tag="shf", bufs=2)
                nc.vector.stream_shuffle(shf[:], qkT[:, m, :], swap_mask)
                t1 = pat.tile([128, T], BF16, tag="t1", bufs=2)
                nc.vector.tensor_mul(t1[:], qkT[:, m, :], cos_sb[:])
                t2 = pat.tile([128, T], BF16, tag="t2", bufs=2)
                nc.vector.tensor_mul(t2[:], shf[:], sin_sb[:])
                nc.vector.tensor_add(qkT[:, m, :], t1[:], t2[:])

        es_store = {}

        def emit_s(i, tkcs):
            qv, kv = i, 6 + i
            for tkc in tkcs:
                lo = 128 * tkc
                width = T - lo
                wide = width > 512
                for hh in range(2):
                    b0 = 64 * hh
                    if wide:
                        st = stw.tile([128, 1024], F32, tag="stw")
                    else:
                        st = ps5.tile([128, 512], F32, tag="ps5")
                    for off in range(0, width, 512):
                        valid_w = min(512, width - off)
                        nc.tensor.matmul(
                            st[:, off : off + valid_w],
                            qkT[b0 : b0 + 64, kv, lo : lo + 128],
                            qkT[b0 : b0 + 64, qv, lo + off : lo + off + valid_w],
                            start=True,
                            stop=not (off == 0),
                        )
                        if off == 0:  # causal mask add on the diagonal block
                            nc.tensor.matmul(
                                st[:, 0:128],
                                idnr_sb,
                                mneg_sb,
                                start=False,
                                stop=True,
                            )
                    if wide:
                        es = pesw.tile([128, 1024], BF16, tag="esw", name="esw")
                    else:
                        es = pes.tile([128, 512], BF16, tag="es", name="es")
                    nc.scalar.activation(
                        es[:, :width], st[:, :width], AFT.Exp, scale=0.125
                    )
                    es_store[(hh, tkc)] = es

        def emit_v():
            for vg in range(2):
                wt = wtv[vg]
                for t in range(NT):
                    ps = ps5.tile([128, 512], F32, tag="ps5")
                    for kc in range(KC):
                        nc.tensor.matmul(
                            ps[:, 0:384],
                            xt_sb[:, kc, t * 128 : (t + 1) * 128],
                            wt[:, kc, :],
                            start=(kc == 0),
                            stop=(kc == KC - 1),
                        )
                    nc.vector.tensor_add(
                        v_sb[:, t, 6 * vg : 6 * vg + 6, 0:64],
                        ps[:, 0:384].rearrange("p (h d) -> p h d", h=6),
                        bvbc_sb[:, vg * 384 : (vg + 1) * 384].rearrange(
                            "p (h d) -> p h d", h=6
                        ),
                    )

        def emit_y(i, pjs=(0, 1)):
            for pj in pjs:
                w0 = 512 * pj
                tkcs = [k for k in range(NT) if 128 * k < w0 + 512]
                for hh in range(2):
                    h = 2 * i + hh
                    yp = ps5.tile([128, 512], F32, tag="ps5")
                    for j, tkc in enumerate(tkcs):
                        lo = 128 * tkc
                        plo = max(w0, lo)
                        wdt = w0 + 512 - plo
                        es = es_store[(hh, tkc)]
                        nc.tensor.matmul(
                            yp[0:65, plo - w0 : plo - w0 + wdt],
                            v_sb[:, tkc, h, :],
                            es[:, plo - lo : plo - lo + wdt],
                            start=(j == 0),
                            stop=(j == len(tkcs) - 1),
                        )
                    # normalize: y = y' / r  (r = row 64 of yp):
                    # r -> SBUF f32r, broadcast to 64 rows on PE, reciprocal
                    # moves it back to SBUF, multiply
                    rs = pbs.tile([128, 512], BF16, tag="rs")
                    nc.scalar.activation(
                        rs[64:65, :], yp[64:65, :], AFT.Identity
                    )
                    rbp = stw.tile([128, 1024], F32, tag="stw")
                    nc.tensor.matmul(
                        rbp[0:64, 0:512],
                        ones_b[64:65, :],
                        rs[64:65, :],
                        start=True,
                        stop=True,
                    )
                    rbf = pbs.tile([64, 512], F32, tag="rbf")
                    nc.vector.reciprocal_approx_fast(
                        out=rbf[:], in_=rbp[0:64, 0:512]
                    )
                    if hh == 0:
                        nc.vector.tensor_mul(
                            yT[0:64, i, w0 : w0 + 512], yp[0:64, :], rbf[:]
                        )
                    else:
                        ys = pbs.tile([64, 512], BF16, tag="ys")
                        nc.vector.tensor_mul(ys[:], yp[0:64, :], rbf[:])
                        nc.sync.dma_start(
                            out=yT[64:128, i, w0 : w0 + 512], in_=ys[:]
                        )

        # ---------------- pipelined qkv + attention ----------------
        if 1 not in phases:
            return
        emit_qk(0)
        emit_rope(0)
        for i in range(1, 7):
            if i + 1 < 6:
                wtp[i + 1] = _load_wa("wtp", 2 + (i + 1), 256)
            if i == 4:
                nc.sync.dma_start(
                    out=wp_sb[:],
                    in_=dr["wp"][:].rearrange("p (kc n) -> p kc n", kc=KC),
                )
            if 2 in phases:
                emit_s(i - 1, range(0, 2))
            if i == 1:
                emit_v()
            if i < 6:
                emit_qk(i)
                emit_rope(i)
            if 2 in phases:
                emit_s(i - 1, range(2, NT))
                emit_y(i - 1)

        if _DEBUG:
            nc.sync.dma_start(
                out=dr["dqk"][:], in_=qkT[:].rearrange("p a b -> p (a b)")
            )
            nc.sync.dma_start(
                out=dr["dv"][:], in_=v_sb[:].rearrange("p a b c -> p (a b c)")
            )
            nc.sync.dma_start(
                out=dr["dyt"][:], in_=yT[:].rearrange("p a b -> p (a b)")
            )

        # ---------------- output projection ----------------
        if 3 not in phases:
            return
        with tc.tile_pool(name="pc_ob", bufs=3) as pco:
            for m in range(NT):
                osb = pco.tile([128, C], F32, tag="ob")
                for piece in range(2):
                    pw_ = slice(piece * 384, (piece + 1) * 384)
                    po = ps5.tile([128, 512], F32, tag="ps5")
                    for kc in range(KC):
                        nc.tensor.matmul(
                            po[:, 0:384],
                            yT[:, kc, m * 128 : (m + 1) * 128],
                            wp_sb[:, kc, pw_],
                            start=(kc == 0),
                            stop=(kc == KC - 1),
                        )
                    nc.vector.tensor_add(osb[:, pw_], po[:, 0:384], bpbc_sb[:, pw_])
                    nc.sync.dma_start(
                        out=dr["out"][m * 128 : (m + 1) * 128, pw_],
                        in_=osb[:, pw_],
                    )


def _build_program(loop_n=None, phases=(1, 2, 3)):
    import concourse.bacc as bacc
    import concourse.tile as tile
    from concourse import mybir

    F32 = mybir.dt.float32
    BF16 = mybir.dt.bfloat16

    nc = bacc.Bacc(None, target_bir_lowering=False, debug=False)

    dr = {
        "xt": nc.dram_tensor("xt", [128, KC * T], BF16, kind="ExternalInput"),
        "wa": nc.dram_tensor("wa", [128, KC * _WATOT], BF16, kind="ExternalInput"),
        "bvbc": nc.dram_tensor("bvbc", [128, C], BF16, kind="ExternalInput"),
        "wp": nc.dram_tensor("wp", [128, KC * C], BF16, kind="ExternalInput"),
        "bpbc": nc.dram_tensor("bpbc", [128, C], F32, kind="ExternalInput"),
        "cbf": nc.dram_tensor("cbf", [128, 2 * T + 320], BF16, kind="ExternalInput"),
        "cf32": nc.dram_tensor("cf32", [128, 12], F32, kind="ExternalInput"),
        "vones": nc.dram_tensor("vones", [128, NT * H], BF16, kind="ExternalInput"),
        "out": nc.dram_tensor("out", [T, C], F32, kind="ExternalOutput"),
    }
    if _DEBUG:
        dr["dqk"] = nc.dram_tensor("dqk", [128, 12 * T], BF16, kind="ExternalOutput")
        dr["dv"] = nc.dram_tensor("dv", [128, NT * H * 65], BF16, kind="ExternalOutput")
        dr["dyt"] = nc.dram_tensor("dyt", [128, KC * T], BF16, kind="ExternalOutput")

    with tile.TileContext(nc) as tc:
        if loop_n is None:
            _emit_body(nc, tc, dr, phases)
        else:
            with tc.For_i(0, loop_n, 1):
                _emit_body(nc, tc, dr, phases)

    nc.compile()
    return nc


def _host_constants():
    """Constant tables shipped to every core."""
    inv_freq = (1.0 / (10000.0 ** (np.arange(0, D, 2, dtype=np.float32) / D))).astype(
        np.float32
    )
    tpos = np.arange(T, dtype=np.float32)
    freqs = tpos[None, :] * inv_freq[:, None]  # [32, T]
    cos32 = np.cos(freqs).astype(np.float32)
    sin32 = np.sin(freqs).astype(np.float32)
    cosT = np.repeat(cos32, 2, axis=0)  # [64, T], channel d -> freq d//2
    sinT = np.repeat(sin32, 2, axis=0)
    cosT = np.concatenate([cosT, cosT], axis=0)  # [128, T]: two head copies
    sinT = np.concatenate([sinT, sinT], axis=0)

    # rotation = adjacent-row swap; fold the signs into the sin table:
    # rot[2i] = -q[2i+1]*sin, rot[2i+1] = +q[2i]*sin
    sinS = sinT.copy()
    sinS[0::2, :] *= -1.0

    import ml_dtypes

    # additive mask in s^T orientation: -1e5 (pre-scale) where tq_rel < tk
    mneg = (-1.0e5 * np.tril(np.ones((128, 128), dtype=np.float32), k=-1)).astype(
        ml_dtypes.bfloat16
    )
    idn_b = np.eye(128, dtype=np.float32).astype(ml_dtypes.bfloat16)
    return cosT, sinS, mneg, idn_b


def _input_maps(x, W_attn, b_attn, W_proj, b_proj):
    import ml_dtypes

    BF = ml_dtypes.bfloat16
    cosT, sinS, mneg_b, idn_b = _host_constants()

    # wa: [C, 3C] -> [128, kc, n], columns packed as [vA | vB | pair0..pair5]
    wa = W_attn.reshape(KC, 128, 3 * C).transpose(1, 0, 2)  # [128, kc, 3C]
    groups = [wa[:, :, 2 * C : 2 * C + 384], wa[:, :, 2 * C + 384 : 3 * C]]
    for hp in range(6):
        groups.append(wa[:, :, hp * 128 : (hp + 1) * 128])  # q pair
        groups.append(wa[:, :, C + hp * 128 : C + (hp + 1) * 128])  # k pair
    # merge each pair's q+k into one 256-col group
    packed = [groups[0], groups[1]] + [
        np.concatenate([groups[2 + 2 * hp], groups[3 + 2 * hp]], axis=2)
        for hp in range(6)
    ]
    wa_g = np.concatenate([g.reshape(128, -1) for g in packed], axis=1)
    assert wa_g.shape[1] == KC * _WATOT

    wp = W_proj.reshape(KC, 128, C).transpose(1, 0, 2).reshape(128, KC * C)

    bqk = b_attn[: 2 * C].reshape(12, 128).T.astype(np.float32)
    cbf = np.concatenate(
        [
            cosT.astype(BF),
            sinS.astype(BF),
            mneg_b,
            idn_b,
            np.ones((128, 64), BF),
        ],
        axis=1,
    )
    cf32 = np.ascontiguousarray(bqk)
    shared = {
        "wa": np.ascontiguousarray(wa_g.astype(BF)),
        "bvbc": np.ascontiguousarray(
            np.broadcast_to(b_attn[2 * C :].astype(BF), (128, C))
        ),
        "wp": np.ascontiguousarray(wp.astype(BF)),
        "bpbc": np.ascontiguousarray(np.broadcast_to(b_proj, (128, C))),
        "cbf": np.ascontiguousarray(cbf),
        "cf32": cf32,
        "vones": np.ones((128, NT * H), dtype=BF),
    }
    out = []
    for b in range(B):
        xt = (
            x[b].T.reshape(KC, 128, T).transpose(1, 0, 2).reshape(128, KC * T)
        )  # [128, KC*T]
        out.append(dict(shared, xt=np.ascontiguousarray(xt.astype(BF))))
    return out


def kernel(x, W_attn, b_attn, W_proj, b_proj):
    global _prog
    from concourse.bass_utils import run_bass_kernel_spmd

    if _prog is None:
        _prog = _build_program()

    x = np.asarray(x, dtype=np.float32)
    W_attn = np.asarray(W_attn, dtype=np.float32)
    b_attn = np.asarray(b_attn, dtype=np.float32)
    W_proj = np.asarray(W_proj, dtype=np.float32)
    b_proj = np.asarray(b_proj, dtype=np.float32)

    in_maps = _input_maps(x, W_attn, b_attn, W_proj, b_proj)
    # first post-load execution shows cold-start wobble in some ucode
    # engines; run once to warm up, return the steady-state result
    run_bass_kernel_spmd(_prog, in_maps, list(range(N_CORES)))
    res = run_bass_kernel_spmd(_prog, in_maps, list(range(N_CORES)))
    out = np.stack([res.results[b]["out"] for b in range(B)], axis=0)
    return out.astype(np.float32)
